# revision 3
# baseline (speedup 1.0000x reference)
"""Masked tanh-clipped dot-product attention on 8 Trainium2 NeuronCores.

Reference computation (per batch b of 16):
    logits = Q @ K^T / sqrt(128)          [2048, 2048]
    logits = 10 * tanh(logits)
    logits[:, masked_n] = -inf            (mask is per-key)
    out = softmax(logits, -1) @ V         [2048, 128]

Sharding: batch dim 16 -> 2 batches per core (pure data parallel).

Device kernel (per core, per batch), computed in the transposed layout
S^T[n, m] so the PV matmul needs no on-chip transposes:
    ST = KT.T @ QT              (f32r matmuls, contraction over d)
    E  = exp(10 * tanh(ST / sqrt(d)))
    OUTT[d, m]   += V[nchunk].T @ E      (V rows pre-zeroed for masked keys)
    ROWSUM[1, m] += valid[nchunk].T @ E  (valid = 1 - mask)
Host divides OUTT by ROWSUM and transposes back. Masked keys therefore
contribute exactly 0 to both numerator and denominator, reproducing the
-inf masking; no max-subtraction is needed because 10*tanh bounds the
logits to [-10, 10].
"""

import sys

for _p in ("/opt/trn_rl_repo", "/root/.axon_site/_ro/trn_rl_repo"):
    if _p not in sys.path:
        sys.path.insert(0, _p)

from contextlib import ExitStack

import numpy as np

import concourse.bacc as bacc
import concourse.bass as bass
import concourse.mybir as mybir
import concourse.tile as tile

F32 = mybir.dt.float32
F32R = mybir.dt.float32r
ActFn = mybir.ActivationFunctionType

N_CORES = 8
B = 16
B_LOC = B // N_CORES  # batches per core
M = 2048              # queries
N = 2048              # keys
D = 128               # head dim
P = 128               # partitions
MF = 512              # matmul moving free dim
MH = 1024             # m half-window (2 matmul chunks, wide ACT ops)
N_CH = N // P         # 16 key chunks
SCALE = float(1.0 / np.sqrt(128.0))
CLIP = 10.0


def _build_nc(reps=1):
    nc = bacc.Bacc("TRN2", target_bir_lowering=False, debug=False)

    qt = nc.dram_tensor("qt", [B_LOC, P, M], F32R, kind="ExternalInput")
    kt = nc.dram_tensor("kt", [B_LOC, P, N], F32R, kind="ExternalInput")
    v = nc.dram_tensor("v", [B_LOC, N_CH, P, D], F32R, kind="ExternalInput")
    valid = nc.dram_tensor("valid", [B_LOC, P, N_CH], F32R, kind="ExternalInput")
    outt = nc.dram_tensor("outt", [B_LOC, D, M], F32, kind="ExternalOutput")
    rs = nc.dram_tensor("rs", [B_LOC, 1, M], F32, kind="ExternalOutput")

    with tile.TileContext(nc) as tc, ExitStack() as outer:
        if reps > 1:
            outer.enter_context(tc.For_i(0, reps, 1))
        with ExitStack() as ctx:
            io_pool = ctx.enter_context(tc.tile_pool(name="io", bufs=2))
            act_pool = ctx.enter_context(tc.tile_pool(name="act", bufs=2))
            e_pool = ctx.enter_context(tc.tile_pool(name="e", bufs=3))
            out_pool = ctx.enter_context(tc.tile_pool(name="out", bufs=2))
            ps_s = ctx.enter_context(tc.tile_pool(name="ps_s", bufs=2, space="PSUM"))
            ps_acc = ctx.enter_context(
                tc.tile_pool(name="ps_acc", bufs=1, space="PSUM")
            )

            for b in range(B_LOC):
                qt_sb = io_pool.tile([P, M], F32R, tag="qt")
                nc.sync.dma_start(qt_sb[:], qt[b])
                kt_sb = io_pool.tile([P, N], F32R, tag="kt")
                nc.sync.dma_start(kt_sb[:], kt[b])
                v_sb = io_pool.tile([P, N_CH, D], F32R, tag="v")
                for ni in range(N_CH):
                    nc.sync.dma_start(v_sb[:, ni, :], v[b, ni])
                valid_sb = io_pool.tile([P, N_CH], F32R, tag="valid")
                nc.sync.dma_start(valid_sb[:], valid[b])

                for mh in range(M // MH):
                    m0 = mh * MH
                    outt_ps = [
                        ps_acc.tile([P, MF], F32, tag=f"outt{k}", name=f"outt_ps{k}")
                        for k in range(2)
                    ]
                    rs_ps = [
                        ps_acc.tile([1, MF], F32, tag=f"rs{k}", name=f"rs_ps{k}")
                        for k in range(2)
                    ]

                    for ni in range(N_CH):
                        s_ps = ps_s.tile([P, MH], F32, tag="s")
                        for k in range(2):
                            nc.tensor.matmul(
                                s_ps[:, k * MF:(k + 1) * MF],
                                kt_sb[:, ni * P:(ni + 1) * P],
                                qt_sb[:, m0 + k * MF:m0 + (k + 1) * MF],
                                start=True, stop=True,
                            )
                        t_sb = act_pool.tile([P, MH], F32, tag="t")
                        nc.scalar.activation(t_sb[:], s_ps[:], ActFn.Tanh, scale=SCALE)
                        e_sb = e_pool.tile([P, MH], F32R, tag="e")
                        nc.scalar.activation(e_sb[:], t_sb[:], ActFn.Exp, scale=CLIP)

                        first, last = ni == 0, ni == N_CH - 1
                        for k in range(2):
                            ek = e_sb[:, k * MF:(k + 1) * MF]
                            nc.tensor.matmul(
                                outt_ps[k][:], v_sb[:, ni, :], ek,
                                start=first, stop=last,
                            )
                            nc.tensor.matmul(
                                rs_ps[k][:], valid_sb[:, ni:ni + 1], ek,
                                start=first, stop=last,
                            )

                    for k in range(2):
                        o_sb = out_pool.tile([P, MF], F32, tag="o")
                        nc.vector.tensor_copy(o_sb[:], outt_ps[k][:])
                        nc.sync.dma_start(
                            outt[b, :, m0 + k * MF:m0 + (k + 1) * MF], o_sb[:]
                        )
                        r_sb = out_pool.tile([1, MF], F32, tag="r")
                        nc.vector.tensor_copy(r_sb[:], rs_ps[k][:])
                        nc.sync.dma_start(
                            rs[b, :, m0 + k * MF:m0 + (k + 1) * MF], r_sb[:]
                        )
    nc.compile()
    return nc


class Runner:
    """Persistent compiled SPMD runner (mirrors bass2jax.run_bass_via_pjrt's
    multi-core path, but keeps the jitted callable across calls)."""

    def __init__(self, reps=1):
        import jax
        from jax.experimental.shard_map import shard_map
        from jax.sharding import Mesh, PartitionSpec
        from concourse.bass2jax import (
            _bass_exec_p,
            install_neuronx_cc_hook,
            partition_id_tensor,
        )

        self._jax = jax
        install_neuronx_cc_hook()
        nc = _build_nc(reps)
        self.nc = nc

        in_names, out_names, out_avals, zero_outs = [], [], [], []
        partition_name = (
            nc.partition_id_tensor.name if nc.partition_id_tensor else None
        )
        for alloc in nc.m.functions[0].allocations:
            if not isinstance(alloc, mybir.MemoryLocationSet):
                continue
            name = alloc.memorylocations[0].name
            if alloc.kind == "ExternalInput":
                if name != partition_name:
                    in_names.append(name)
            elif alloc.kind == "ExternalOutput":
                out_names.append(name)
                shape = tuple(alloc.tensor_shape)
                dtype = mybir.dt.np(alloc.dtype)
                out_avals.append(jax.core.ShapedArray(shape, dtype))
                zero_outs.append(np.zeros(shape, dtype))
        self.in_names = list(in_names)
        self.out_names = out_names
        self.out_avals = out_avals
        self.zero_outs = zero_outs
        n_params = len(in_names)
        n_outs = len(out_names)
        all_in_names = in_names + out_names
        if partition_name is not None:
            all_in_names.append(partition_name)

        def _body(*args):
            operands = list(args)
            if partition_name is not None:
                operands.append(partition_id_tensor())
            return tuple(_bass_exec_p.bind(
                *operands,
                out_avals=tuple(out_avals),
                in_names=tuple(all_in_names),
                out_names=tuple(out_names),
                lowering_input_output_aliases=(),
                sim_require_finite=True,
                sim_require_nnan=True,
                nc=nc,
            ))

        devices = jax.devices()[:N_CORES]
        self.mesh = Mesh(np.asarray(devices), ("core",))
        in_specs = (PartitionSpec("core"),) * (n_params + n_outs)
        out_specs = (PartitionSpec("core"),) * n_outs
        self.sharded = jax.jit(
            shard_map(_body, mesh=self.mesh, in_specs=in_specs,
                      out_specs=out_specs, check_rep=False),
            donate_argnums=tuple(range(n_params, n_params + n_outs)),
            keep_unused=True,
        )

    def put_inputs(self, in_maps):
        return [
            np.concatenate(
                [np.asarray(in_maps[c][n]) for c in range(N_CORES)], axis=0
            )
            for n in self.in_names
        ]

    def __call__(self, concat_in):
        zeros = [
            np.zeros((N_CORES * z.shape[0], *z.shape[1:]), z.dtype)
            for z in self.zero_outs
        ]
        out_arrs = self.sharded(*concat_in, *zeros)
        out_arrs = [np.asarray(a) for a in out_arrs]
        return [
            {
                name: out_arrs[i].reshape(N_CORES, *self.out_avals[i].shape)[c]
                for i, name in enumerate(self.out_names)
            }
            for c in range(N_CORES)
        ]


_RUNNER = None


def _get_runner():
    global _RUNNER
    if _RUNNER is None:
        _RUNNER = Runner()
    return _RUNNER


def _prep_in_maps(Q, K, V, mask):
    Q = np.asarray(Q, dtype=np.float32)
    K = np.asarray(K, dtype=np.float32)
    V = np.asarray(V, dtype=np.float32)
    mask = np.asarray(mask)
    valid = (~mask.astype(bool)).astype(np.float32)[:, :, 0]   # [B, N]

    qt = np.ascontiguousarray(Q.transpose(0, 2, 1))            # [B, D, M]
    kt = np.ascontiguousarray(K.transpose(0, 2, 1))            # [B, D, N]
    vm = np.ascontiguousarray(
        (V * valid[:, :, None]).reshape(B, N_CH, P, D)         # [B, nch, p, d]
    )
    # valid_t[b][i, ni] = valid[b, ni*128 + i]
    valid_t = np.ascontiguousarray(
        valid.reshape(B, N_CH, P).transpose(0, 2, 1)           # [B, P, nch]
    )

    in_maps = []
    for c in range(N_CORES):
        s = slice(c * B_LOC, (c + 1) * B_LOC)
        in_maps.append({
            "qt": np.ascontiguousarray(qt[s]),
            "kt": np.ascontiguousarray(kt[s]),
            "v": np.ascontiguousarray(vm[s]),
            "valid": np.ascontiguousarray(valid_t[s]),
        })
    return in_maps


def _postprocess(results):
    out = np.empty((B, M, D), dtype=np.float32)
    for c in range(N_CORES):
        outt = results[c]["outt"]                              # [B_LOC, D, M]
        rsum = results[c]["rs"][:, 0, :]                       # [B_LOC, M]
        for b in range(B_LOC):
            out[c * B_LOC + b] = outt[b].T / rsum[b][:, None]
    return out


def kernel(Q, K, V, mask):
    runner = _get_runner()
    results = runner(runner.put_inputs(_prep_in_maps(Q, K, V, mask)))
    return _postprocess(results)
